# revision 56
# baseline (speedup 1.0000x reference)
"""Trainium2 Bass kernel for nn_Encoder_17824114278582.

Strategy (v2):
- Data-parallel over batch B=8 across 8 NeuronCores (1 batch elem / core).
- Host-side: fold LayerNorm gamma/beta + softmax scale into the linear
  weights; pack ALL weights into one [128, 2048] bf16 DRAM tensor (1 DMA).
  Non-ones mask / non-zero biases fall back to an exact numpy path.
- On-device per layer, engine-balanced + software-pipelined:
    LN (bn_stats/aggr DVE, rstd ACT per-half, xhat Pool, PE transpose,
    psum->sbuf copies DVE 4x-mode)
    eop: fused [d,384] matmul -> relu (Pool) -> 3-way reduce (DVE)
    qT/kT via W-stationary matmuls (ACT copies); v via hT-stationary
    attention (transposed scores), per tj-pair [128,2048] granularity:
       scores (PE) -> exp (ACT, fp16) -> mask = e>=c (DVE tensor_scalar,
       4x mode) -> p = e*mask (DVE tensor_tensor, 2x mode)
       att_acc += v @ p ; rs += ones @ p  (PE, lagged 2 pairs for overlap)
    rowsum -> partition form via DMA fold; att^T -> natural via per-tile
    DMA transposes (frees PE/PSUM); fixup r = att*recip + s (Pool STT)
    FFN: W-stationary + relu (ACT) -> mT; ffn2 + residual (Pool STT)
- All input/output DMAs batched and issued on the idle SP (sync) queue.
"""
import sys
for _p in ("/opt/trn_rl_repo", "/root/.axon_site/_ro/trn_rl_repo"):
    if _p not in sys.path:
        sys.path.insert(0, _p)

import math
from contextlib import ExitStack

import numpy as np
import ml_dtypes

import concourse.bass as bass
import concourse.tile as tile
from concourse import mybir
from concourse.bass_utils import run_bass_kernel_spmd

F32 = mybir.dt.float32
BF16 = mybir.dt.bfloat16
F16 = mybir.dt.float16
F8 = mybir.dt.float8e4
AF = mybir.ActivationFunctionType
OP = mybir.AluOpType
DR = mybir.MatmulPerfMode.DoubleRow

B, S, DIM = 8, 2048, 128
L = 2
HEAD_SIZE = 32
NT = S // 128
LN_EPS = 1e-12
THRESH = 1e-3
CPRIME = float(np.float16(np.exp(np.float32(THRESH))))

# column offsets inside the packed weight tensor [128, 2048]
OFF_EOP = 0     # li*384
OFF_Q = 768     # li*128
OFF_K = 1024
OFF_V = 1280
OFF_1 = 1536
OFF_2 = 1792

_BUILD_CACHE = {}


def _split_multi_waits(nc, max_waits=1):
    """walrus on this stack rejects instructions carrying more than one
    sync-wait command.  Hoist surplus waits onto same-engine NoOps inserted
    directly before the instruction (queue order preserves semantics)."""
    nop_id = [0]
    for fn in nc.m.functions:
        for blk in fn.blocks:
            out = []
            for ins in blk.instructions:
                si = ins.sync_info
                waits = list(si.on_wait) if si is not None and si.on_wait else []
                limit = max_waits
                if type(ins).__name__ in ("InstDmaTransposeAnt",):
                    limit = 0
                if len(waits) > limit:
                    keep = waits[len(waits) - limit:] if limit else []
                    for w in waits[:len(waits) - limit]:
                        nop = mybir.InstNoOp(
                            name=f"I-waitnop-{nop_id[0]}", ins=[], outs=[])
                        nop_id[0] += 1
                        nop.engine = ins.engine
                        nop.sync_info = mybir.SyncInfo(on_wait=[w], on_update=[])
                        out.append(nop)
                    ins.sync_info = mybir.SyncInfo(
                        on_wait=keep, on_update=list(si.on_update or []))
                out.append(ins)
            blk.instructions = out


def _build_encoder(split_waits=True):
    nc = bass.Bass()
    ts = bass.ts

    x_in = nc.declare_dram_parameter("x", [S, DIM], F32, isOutput=False)
    w_in = nc.declare_dram_parameter("w_pack", [DIM, 2048], BF16, isOutput=False)
    out_d = nc.declare_dram_parameter("out", [S, DIM], F32, isOutput=True)

    with tile.TileContext(nc) as tc, ExitStack() as ctx:
        singles = ctx.enter_context(tc.tile_pool(name="singles", bufs=1))
        actp = ctx.enter_context(tc.tile_pool(name="actp", bufs=2))
        sm2 = ctx.enter_context(tc.tile_pool(name="sm2", bufs=2))
        sm3 = ctx.enter_context(tc.tile_pool(name="sm3", bufs=3))
        sm5 = ctx.enter_context(tc.tile_pool(name="sm5", bufs=5))
        psA = ctx.enter_context(tc.tile_pool(name="psA", bufs=2, space="PSUM"))
        psB = ctx.enter_context(tc.tile_pool(name="psB", bufs=1, space="PSUM"))
        psC = ctx.enter_context(tc.tile_pool(name="psC", bufs=2, space="PSUM"))

        # ---- constants ----
        ident_bf = singles.tile([128, 128], BF16)
        nc.gpsimd.memset(ident_bf[:], 0.0)
        nc.gpsimd.affine_select(
            out=ident_bf[:], in_=ident_bf[:], compare_op=OP.not_equal,
            fill=1.0, base=0, pattern=[[-1, 128]], channel_multiplier=1)
        ones16 = singles.tile([128, 1], F16)
        nc.vector.memset(ones16[:], 1.0)
        ident1_f32 = singles.tile([1, 1], F32)
        nc.vector.memset(ident1_f32[:], 1.0)
        eps_t = singles.tile([128, 1], F32)
        nc.vector.memset(eps_t[:], LN_EPS)
        zero_t = singles.tile([128, 1], F32)
        nc.vector.memset(zero_t[:], 0.0)

        # ---- load weights (1 DMA, scalar queue) and x (2 DMAs, sync) ----
        w_sb = singles.tile([128, 2048], BF16)
        nc.scalar.dma_start(w_sb[:], w_in[:, :])
        x_r = x_in.rearrange("(i p) d -> p i d", p=128)
        h_all = actp.tile([128, NT, DIM], F32, tag="h", name="h0")
        for g in range(4):
            nc.sync.dma_start(h_all[:, 4 * g:4 * g + 4, :],
                              x_r[:, 4 * g:4 * g + 4, :])

        def w_slice(off, li, width):
            return w_sb[:, off + li * width:off + (li + 1) * width]

        class LN:
            """LayerNorm pipeline: stats fused into producer loops, finish()
            per 8-tile half emits rstd/xhat/transposes/copies."""
            def __init__(self, tag):
                self.tag = tag
                self.mv = sm2.tile([128, NT, 2], F32, tag="ln_mv",
                                   name=tag + "_mv")
                self.rstd = sm2.tile([128, NT], F32, tag="ln_rstd",
                                     name=tag + "_rstd")
                self.xh = sm2.tile([128, NT, DIM], BF16, tag="ln_xh",
                                   name=tag + "_xh")
                self.xT = actp.tile([128, S], BF16, tag="ln_xT",
                                    name=tag + "_xT")

            def stat(self, i, src_ap):
                st6 = sm3.tile([128, 6], F32, tag="ln_st6",
                               name=f"{self.tag}_st6_{i}")
                nc.vector.bn_stats(st6[:], src_ap)
                nc.vector.bn_aggr(self.mv[:, i, :], st6[:])

            def finish_q(self, g, h_src):
                """finish one quarter (4 tiles = one transpose group)."""
                lnv = sm3.tile([128, 4], F32, tag="ln_lnv",
                               name=f"{self.tag}_lnv_{g}")
                nc.scalar.activation(lnv[:], self.mv[:, ts(g, 4), 1],
                                     AF.Ln, bias=eps_t[:], scale=1.0)
                nc.scalar.activation(self.rstd[:, ts(g, 4)], lnv[:],
                                     AF.Exp, bias=zero_t[:], scale=-0.5)
                for i in range(4 * g, 4 * g + 4):
                    nc.gpsimd.tensor_scalar(
                        out=self.xh[:, i, :], in0=h_src[:, i, :],
                        scalar1=self.mv[:, i, 0:1],
                        scalar2=self.rstd[:, i:i + 1],
                        op0=OP.subtract, op1=OP.mult)
                tr = psA.tile([128, 512], BF16, tag="psA",
                              name=f"{self.tag}_tr_{g}")
                for j in range(4):
                    nc.tensor.transpose(tr[:, ts(j, 128)],
                                        self.xh[:, 4 * g + j, :],
                                        ident_bf[:])
                nc.vector.tensor_copy(self.xT[:, ts(g, 512)], tr[:])

            def finish(self, half, h_src):
                self.finish_q(2 * half, h_src)
                self.finish_q(2 * half + 1, h_src)

        # ---------------- layers ----------------
        ln1 = LN("ln1_0")
        for i in range(NT):
            ln1.stat(i, h_all[:, i, :])

        for li in range(L):
            # ===== LN1 finish + edge ops (+ fused LN2 stats), half-wise =====
            xT = ln1.xT
            ln2 = LN(f"ln2_{li}")
            s_all = actp.tile([128, NT, DIM], F32, tag="s", name=f"s_{li}")

            def eop_tile(i, li=li, ln2=ln2, s_all=s_all, xT=xT):
                f_ps = psC.tile([128, 3 * DIM], F32, tag="psC",
                                name=f"f_ps_{li}_{i}")
                nc.tensor.matmul(f_ps[:], xT[:, ts(i, 128)],
                                 w_slice(OFF_EOP, li, 384),
                                 start=True, stop=True)
                f_rl = sm2.tile([128, 3 * DIM], BF16, tag="f_rl",
                                name=f"f_rl_{li}_{i}")
                nc.scalar.activation(f_rl[:], f_ps[:], AF.Relu,
                                     bias=zero_t[:], scale=1.0)
                if i % 2 == 0:
                    s01 = sm3.tile([128, DIM], BF16, tag="s01",
                                   name=f"s01_{li}_{i}")
                    nc.gpsimd.tensor_tensor(out=s01[:], in0=f_rl[:, 0:128],
                                            in1=f_rl[:, 128:256], op=OP.add)
                    nc.gpsimd.tensor_tensor(out=s_all[:, i, :], in0=s01[:],
                                            in1=f_rl[:, 256:384], op=OP.add)
                else:
                    nc.vector.tensor_reduce(
                        s_all[:, i, :],
                        f_rl[:].rearrange("p (j e) -> p e j", j=3),
                        axis=mybir.AxisListType.X, op=OP.add)
                ln2.stat(i, s_all[:, i, :])

            hT = ln2.xT
            q8 = actp.tile([128, S], F8, tag="q8", name=f"q8_{li}")
            k8 = actp.tile([128, S], F8, tag="k8", name=f"k8_{li}")
            v16 = actp.tile([128, S], F16, tag="v16", name=f"v16_{li}")
            # folded [64, 2, S] (partitions 64-127 into plane 1) so the
            # scores matmul can run in fp8 DoubleRow mode (2 K-planes)
            q8p = actp.tile([64, 2, S], F8, tag="q8p", name=f"q8p_{li}")
            k8p = actp.tile([64, 2, S], F8, tag="k8p", name=f"k8p_{li}")

            def qk_one(nm, hb):
                dst, dstp, off = ((q8, q8p, OFF_Q) if nm == "q"
                                  else (k8, k8p, OFF_K))
                qk_ps = psA.tile([128, 1024], F32, tag="psA",
                                 name=f"qk_{nm}_{li}_{hb}")
                for b in range(2):
                    nc.tensor.matmul(
                        qk_ps[:, ts(b, 512)], w_slice(off, li, 128),
                        hT[:, hb * 1024 + b * 512:hb * 1024 + (b + 1) * 512],
                        start=True, stop=True)
                nc.scalar.activation(dst[:, ts(hb, 1024)], qk_ps[:],
                                     AF.Copy, bias=0.0, scale=1.0)
                nc.sync.dma_start(dstp[:, 0, ts(hb, 1024)],
                                  dst[0:64, ts(hb, 1024)])
                nc.sync.dma_start(dstp[:, 1, ts(hb, 1024)],
                                  dst[64:128, ts(hb, 1024)])

            def v_tile(i):
                v_ps = psC.tile([128, DIM], F32, tag="psC",
                                name=f"v_ps_{li}_{i}")
                nc.tensor.matmul(v_ps[:], hT[:, ts(i, 128)],
                                 w_slice(OFF_V, li, 128), start=True, stop=True)
                nc.vector.tensor_copy(v16[:, ts(i, 128)], v_ps[:])

            ln1.finish(0, h_all)
            for i in range(8):
                eop_tile(i)
                if i == 0:
                    ln1.finish(1, h_all)
            ln2.finish(0, s_all)
            qk_one("q", 0)
            qk_one("k", 0)
            for i in range(8, NT):
                eop_tile(i)

            # ===== attention =====
            r_all = actp.tile([128, NT, DIM], F32, tag="r", name=f"r_{li}")
            ln3 = LN(f"ln3_{li}")

            hb_state = {}

            def emit_att(hb, pj, p16, att_acc, rs, which="both", base=0):
                for r in range(2):
                    tj = 2 * pj + r
                    for b in range(2):
                        o = base + r * 1024 + b * 512
                        mv = p16[:, o:o + 512]
                        if which in ("both", "att"):
                            nc.tensor.matmul(att_acc[:, ts(b, 512)],
                                             v16[:, ts(tj, 128)], mv,
                                             start=(tj == 0),
                                             stop=(tj == NT - 1))
                        if which in ("both", "rs"):
                            nc.tensor.matmul(rs[b][:], ones16[:], mv,
                                             start=(tj == 0),
                                             stop=(tj == NT - 1))

            def post_hb_a(hb):
                """att transposes via DMA + rowsum -> recip (PE transpose)."""
                att_acc, rs = hb_state[hb]
                attT = sm2.tile([128, 1024], BF16, tag="attT",
                                name=f"attT_{li}_{hb}")
                if hb == 0:
                    nc.vector.tensor_copy(attT[:, 0:512], att_acc[:, 0:512])
                    nc.vector.tensor_copy(attT[:, 512:1024],
                                          att_acc[:, 512:1024])
                else:
                    nc.scalar.activation(attT[:, 0:512], att_acc[:, 0:512],
                                         AF.Copy, bias=0.0, scale=1.0)
                    nc.scalar.activation(attT[:, 512:1024],
                                         att_acc[:, 512:1024],
                                         AF.Copy, bias=0.0, scale=1.0)
                att_nat = sm2.tile([128, 8, 128], BF16, tag="att_nat",
                                   name=f"att_nat_{li}_{hb}")
                for k in range(8):
                    nc.sync.dma_start_transpose(att_nat[:, k, :],
                                                attT[:, ts(k, 128)])
                rs_sb = sm3.tile([1, 1024], F32, tag="rs_sb",
                                 name=f"rs_sb_{li}_{hb}")
                for b in range(2):
                    nc.scalar.activation(rs_sb[:, ts(b, 512)], rs[b][:],
                                         AF.Copy, bias=0.0, scale=1.0)
                rsT_ps = psC.tile([128, 8], F32, tag="psC",
                                  name=f"rsT_{li}_{hb}")
                for k in range(8):
                    nc.tensor.transpose(rsT_ps[:, k:k + 1],
                                        rs_sb[0:1, ts(k, 128)], ident1_f32[:])
                recip = sm3.tile([128, 8], F32, tag="recip",
                                 name=f"recip_{li}_{hb}")
                nc.vector.reciprocal(recip[:], rsT_ps[:])
                hb_state[hb] = (att_nat, recip)

            def post_hb_b(hb, klo, khi):
                """fixup r = att*recip + s (Pool) + fused LN3 stats."""
                att_nat, recip = hb_state[hb]
                for k in range(klo, khi):
                    i = hb * 8 + k
                    nc.vector.scalar_tensor_tensor(
                        out=r_all[:, i, :], in0=att_nat[:, k, :],
                        scalar=recip[:, k:k + 1], in1=s_all[:, i, :],
                        op0=OP.mult, op1=OP.add)
                    ln3.stat(i, r_all[:, i, :])

            def _fill_hb0_p0():
                for i in range(0, 8):
                    v_tile(i)

            def _fill_hb0_p1():
                ln2.finish(1, s_all)
                qk_one("k", 1)

            def _fill_hb0_p2():
                qk_one("q", 1)

            def _fill_hb0_p3():
                for i in range(8, NT):
                    v_tile(i)

            extras = {
                0: {0: _fill_hb0_p0, 1: _fill_hb0_p1,
                    2: _fill_hb0_p2, 3: _fill_hb0_p3},
                1: {0: lambda: post_hb_a(0),
                    2: lambda: post_hb_b(0, 0, 2),
                    3: lambda: post_hb_b(0, 2, 4),
                    4: lambda: post_hb_b(0, 4, 6),
                    5: lambda: (post_hb_b(0, 6, 8),
                                ln3.finish_q(0, r_all)),
                    6: lambda: ln3.finish_q(1, r_all)},
            }
            for hb in range(2):
                att_acc = None
                rs = None
                pend_att = []
                pend_rs = []
                for pj in range(8):
                    e16 = sm2.tile([128, 2048], F16, tag="e16",
                                   name=f"e16_{li}_{hb}_{pj}")
                    for r in range(2):
                        tj = 2 * pj + r
                        sc = psA.tile([128, 1024], F32, tag="psA",
                                      name=f"sc_{li}_{hb}_{tj}")
                        for b in range(2):
                            nc.tensor.matmul(
                                sc[:, ts(b, 512)], k8p[:, :, ts(tj, 128)],
                                q8p[:, :, hb * 1024 + b * 512:
                                    hb * 1024 + (b + 1) * 512],
                                start=True, stop=True, perf_mode=DR)
                        nc.scalar.activation(e16[:, ts(r, 1024)], sc[:],
                                             AF.Exp, bias=zero_t[:], scale=1.0)
                    m16 = sm2.tile([128, 2048], F16, tag="m16",
                                   name=f"m16_{li}_{hb}_{pj}")
                    nc.vector.tensor_scalar(out=m16[:], in0=e16[:],
                                            scalar1=CPRIME, scalar2=None,
                                            op0=OP.is_ge)
                    p16 = sm5.tile([128, 2048], F16, tag="p16",
                                   name=f"p16_{li}_{hb}_{pj}")
                    nc.vector.tensor_tensor(out=p16[:], in0=e16[:],
                                            in1=m16[:], op=OP.mult)
                    pend_att.append((pj, p16, 0))
                    pend_rs.append((pj, p16, 0))
                    while len(pend_att) > 2:
                        if att_acc is None:
                            att_acc = psB.tile([128, 1024], F32, tag="att",
                                               name=f"att_{li}_{hb}")
                        j, pt, ba = pend_att.pop(0)
                        emit_att(hb, j, pt, att_acc, None, which="att", base=ba)
                    while len(pend_rs) > 4:
                        if rs is None:
                            rs = [psC.tile([1, 512], F32, tag="psC",
                                           name=f"rs_{li}_{hb}_{b}")
                                  for b in range(2)]
                        j, pt, ba = pend_rs.pop(0)
                        emit_att(hb, j, pt, None, rs, which="rs", base=ba)
                    # interleaved fill work (v/q projections, prev-hb post)
                    fn = extras[hb].get(pj)
                    if fn is not None:
                        fn()
                # drain: att first (its stop gates the attT copy), then rs
                for j, pt, ba in pend_att:
                    emit_att(hb, j, pt, att_acc, None, which="att", base=ba)
                for j, pt, ba in pend_rs:
                    emit_att(hb, j, pt, None, rs, which="rs", base=ba)
                hb_state[hb] = (att_acc, rs)

            # ===== FFN (interleaved with hb1 post-processing) =====
            gT = ln3.xT
            mT = actp.tile([128, S], BF16, tag="mT", name=f"mT_{li}")
            last = (li == L - 1)
            new_h = actp.tile([128, NT, DIM], F32, tag="h",
                              name=f"h{li + 1}")
            if not last:
                ln_next = LN(f"ln1_{li + 1}")
            out_r = out_d.rearrange("(i p) d -> p i d", p=128)

            def ffn1_half(hb):
                m_ps = psA.tile([128, 1024], F32, tag="psA",
                                name=f"m_ps_{li}_{hb}")
                for b in range(2):
                    nc.tensor.matmul(
                        m_ps[:, ts(b, 512)], w_slice(OFF_1, li, 128),
                        gT[:, hb * 1024 + b * 512:hb * 1024 + (b + 1) * 512],
                        start=True, stop=True)
                nc.scalar.activation(mT[:, ts(hb, 1024)], m_ps[:],
                                     AF.Relu, bias=zero_t[:], scale=1.0)

            def ffn1_chunk(hb, b):
                m_ps = psC.tile([128, 512], F32, tag="psC",
                                name=f"m_ps_{li}_{hb}_{b}")
                nc.tensor.matmul(
                    m_ps[:], w_slice(OFF_1, li, 128),
                    gT[:, hb * 1024 + b * 512:hb * 1024 + (b + 1) * 512],
                    start=True, stop=True)
                nc.scalar.activation(mT[:, hb * 1024 + b * 512:
                                        hb * 1024 + (b + 1) * 512], m_ps[:],
                                     AF.Relu, bias=zero_t[:], scale=1.0)

            def ffn2_tile(i):
                h2_ps = psC.tile([128, DIM], F32, tag="psC",
                                 name=f"h2_ps_{li}_{i}")
                nc.tensor.matmul(h2_ps[:], mT[:, ts(i, 128)],
                                 w_slice(OFF_2, li, 128), start=True, stop=True)
                nc.vector.scalar_tensor_tensor(
                    out=new_h[:, i, :], in0=h2_ps[:], scalar=0.0,
                    in1=r_all[:, i, :], op0=OP.bypass, op1=OP.add)
                if not last:
                    ln_next.stat(i, new_h[:, i, :])
                elif i % 4 == 3:
                    nc.sync.dma_start(out_r[:, i - 3:i + 1, :],
                                      new_h[:, i - 3:i + 1, :])

            ffn1_half(0)
            post_hb_a(1)
            for i in range(0, 4):
                ffn2_tile(i)
            post_hb_b(1, 0, 4)
            for i in range(4, 8):
                ffn2_tile(i)
            post_hb_b(1, 4, 8)
            ln3.finish_q(2, r_all)
            ffn1_chunk(1, 0)
            for i in range(8, 12):
                ffn2_tile(i)
            ln3.finish_q(3, r_all)
            ffn1_chunk(1, 1)
            for i in range(12, NT):
                ffn2_tile(i)
            if not last:
                h_all = new_h
                ln1 = ln_next

    if split_waits:
        _split_multi_waits(nc)
    return nc


def _fold_weights(inputs):
    """Fold LN gamma/beta and softmax scale into the linear weights (fp32)."""
    g = {k: np.asarray(v, np.float32) for k, v in inputs.items()}
    scale = 1.0 / math.sqrt(HEAD_SIZE)
    Wp_eop = np.einsum("lod,lode->lode", g["eop_ln_w"], g["eop_W"])
    bp_eop = np.einsum("lod,lode->loe", g["eop_ln_b"], g["eop_W"]) + g["eop_b"]
    Wp_q = np.einsum("ld,lde->lde", g["attn_ln_w"], g["Wq"]) * scale
    bp_q = (np.einsum("ld,lde->le", g["attn_ln_b"], g["Wq"]) + g["bq"]) * scale
    Wp_k = np.einsum("ld,lde->lde", g["attn_ln_w"], g["Wk"])
    bp_k = np.einsum("ld,lde->le", g["attn_ln_b"], g["Wk"]) + g["bk"]
    Wp_v = np.einsum("ld,lde->lde", g["attn_ln_w"], g["Wv"])
    bp_v = np.einsum("ld,lde->le", g["attn_ln_b"], g["Wv"]) + g["bv"]
    Wp_1 = np.einsum("ld,lde->lde", g["ffn_ln_w"], g["W1"])
    bp_1 = np.einsum("ld,lde->le", g["ffn_ln_b"], g["W1"]) + g["b1"]
    biases = [bp_eop, bp_q, bp_k, bp_v, bp_1, g["b2"]]
    w_eop_f = np.concatenate([Wp_eop[:, o] for o in range(3)], axis=-1)
    return (w_eop_f, Wp_q, Wp_k, Wp_v, Wp_1, g["W2"]), biases


def _pack_weights(inputs):
    """Pack all folded weights into one [128, 2048] bf16 array."""
    (w_eop_f, Wp_q, Wp_k, Wp_v, Wp_1, W2), biases = _fold_weights(inputs)
    w = np.zeros((DIM, 2048), np.float32)
    for li in range(L):
        w[:, OFF_EOP + li * 384:OFF_EOP + (li + 1) * 384] = w_eop_f[li]
        w[:, OFF_Q + li * 128:OFF_Q + (li + 1) * 128] = Wp_q[li]
        w[:, OFF_K + li * 128:OFF_K + (li + 1) * 128] = Wp_k[li]
        w[:, OFF_V + li * 128:OFF_V + (li + 1) * 128] = Wp_v[li]
        w[:, OFF_1 + li * 128:OFF_1 + (li + 1) * 128] = Wp_1[li]
        w[:, OFF_2 + li * 128:OFF_2 + (li + 1) * 128] = W2[li]
    return np.ascontiguousarray(w.astype(ml_dtypes.bfloat16)), biases


def _device_inputs(inputs):
    """Per-core name->array maps for the device kernel (fast path only)."""
    w_pack, _ = _pack_weights(inputs)
    x = np.asarray(inputs["x"], np.float32)
    shared = {"w_pack": w_pack}
    return [dict(shared, x=np.ascontiguousarray(x[b])) for b in range(B)]


def _numpy_fallback(inputs):
    """Exact (fp32) host implementation for inputs outside the fast path."""
    ARCH = [[0, 0, 0, 0, 1], [0, 1, 0, 0, 1]]
    g = {k: np.asarray(v, np.float32) for k, v in inputs.items()}
    scale = 1.0 / math.sqrt(HEAD_SIZE)

    def ln(x, w, b):
        u = x.mean(-1, keepdims=True)
        s = ((x - u) ** 2).mean(-1, keepdims=True)
        return w * ((x - u) / np.sqrt(s + LN_EPS)) + b

    def edge(h, li, oi):
        h = ln(h, g["eop_ln_w"][li, oi], g["eop_ln_b"][li, oi])
        return np.maximum(h @ g["eop_W"][li, oi] + g["eop_b"][li, oi], 0.0)

    xs = [g["x"]]
    for i, (o1, prev, o2, o3, n) in enumerate(ARCH):
        s = edge(xs[i], i, 0) + edge(xs[prev], i, 1) + edge(xs[prev], i, 2)
        h = ln(s, g["attn_ln_w"][i], g["attn_ln_b"][i])
        q = h @ g["Wq"][i] + g["bq"][i]
        k = h @ g["Wk"][i] + g["bk"][i]
        v = h @ g["Wv"][i] + g["bv"][i]
        sc = np.einsum("bsd,btd->bst", q, k) * g["mask"] * scale
        sc = np.where(sc < THRESH, np.float32(-10000.0), sc).astype(np.float32)
        sc -= sc.max(axis=2, keepdims=True)
        p = np.exp(sc)
        p /= p.sum(axis=2, keepdims=True)
        att = np.einsum("bst,btd->bsd", p, v) + s
        h2 = ln(att, g["ffn_ln_w"][i], g["ffn_ln_b"][i])
        h2 = np.maximum(h2 @ g["W1"][i] + g["b1"][i], 0.0)
        h2 = h2 @ g["W2"][i] + g["b2"][i]
        xs.append(h2 + att)
    return xs[-1].astype(np.float32)


_LAST_RESULTS = {}


def kernel(**inputs):
    mask = np.asarray(inputs["mask"])
    _, biases = _fold_weights(inputs)

    fast = bool(np.all(mask == 1.0)) and all(
        float(np.abs(b).max()) == 0.0 for b in biases)
    if not fast:
        return _numpy_fallback(inputs)

    if "nc" not in _BUILD_CACHE:
        _BUILD_CACHE["nc"] = _build_encoder()
    nc = _BUILD_CACHE["nc"]

    in_maps = _device_inputs(inputs)
    res = run_bass_kernel_spmd(nc, in_maps, core_ids=list(range(B)),
                               trace=_LAST_RESULTS.get("trace", False))
    _LAST_RESULTS["results"] = res
    return np.stack([res.results[b]["out"] for b in range(B)], axis=0)


# revision 64
# speedup vs baseline: 1.1044x; 1.1044x over previous
"""Trainium2 Bass kernel for nn_Encoder_17824114278582.

Strategy (v2):
- Data-parallel over batch B=8 across 8 NeuronCores (1 batch elem / core).
- Host-side: fold LayerNorm gamma/beta + softmax scale into the linear
  weights; pack ALL weights into one [128, 2048] bf16 DRAM tensor (1 DMA).
  Non-ones mask / non-zero biases fall back to an exact numpy path.
- On-device per layer, engine-balanced + software-pipelined (Pool/GPSIMD
  cannot read PSUM and cannot run scalar_tensor_tensor, so all PSUM
  consumers sit on ACT/DVE; Pool gets SBUF-side work only):
    LN (bn_stats/aggr DVE, rstd ACT per-quarter, xhat Pool, PE transpose,
    psum->sbuf copies DVE 4x-mode), pipelined at 4-tile granularity
    eop: fused [d,384] matmul -> relu (ACT) -> 3-way sum (Pool adds /
    DVE reduce alternating)
    q/k projected to fp8, DMA-folded [128,S]->[64,2,S] so the SxS scores
    matmul runs in fp8 DoubleRow mode (2 K-planes, 2x PE throughput);
    v via hT-stationary matmuls (DVE copies), emitted inside the hb0
    attention loop as fill work
    attention (transposed scores), per tj-pair [128,2048] granularity:
       scores (PE fp8 DoubleRow) -> exp (ACT, fp16) -> mask = e>=c (DVE
       tensor_scalar imm, 4x mode) -> p = e*mask (DVE tensor_tensor, 2x)
       att_acc += v @ p (PE, lag 2 pairs); rs += ones @ p (PE, lag 4)
    rowsum -> partition form via PE transpose + DVE recip; att^T ->
    natural via per-tile DMA transposes (frees PE/PSUM); fixup
    r = att*recip + s (DVE STT); prev-hb post-processing and the FFN
    halves are interleaved into the next attention loop / layer tail
    FFN: W-stationary + relu (ACT) -> mT; ffn2 + residual (DVE STT)
- All input/output/fold/transpose DMAs batched on the idle SP queue.
- Measured (TimelineSim single-core): 258us (v1 baseline) -> 191us.
"""
import sys
for _p in ("/opt/trn_rl_repo", "/root/.axon_site/_ro/trn_rl_repo"):
    if _p not in sys.path:
        sys.path.insert(0, _p)

import math
from contextlib import ExitStack

import numpy as np
import ml_dtypes

import concourse.bass as bass
import concourse.tile as tile
from concourse import mybir
from concourse.bass_utils import run_bass_kernel_spmd

F32 = mybir.dt.float32
BF16 = mybir.dt.bfloat16
F16 = mybir.dt.float16
F8 = mybir.dt.float8e4
AF = mybir.ActivationFunctionType
OP = mybir.AluOpType
DR = mybir.MatmulPerfMode.DoubleRow

B, S, DIM = 8, 2048, 128
L = 2
HEAD_SIZE = 32
NT = S // 128
LN_EPS = 1e-12
THRESH = 1e-3
CPRIME = float(np.float16(np.exp(np.float32(THRESH))))

# column offsets inside the packed weight tensor [128, 2048]
OFF_EOP = 0     # li*384
OFF_Q = 768     # li*128
OFF_K = 1024
OFF_V = 1280
OFF_1 = 1536
OFF_2 = 1792

_BUILD_CACHE = {}


def _split_multi_waits(nc, max_waits=1):
    """walrus on this stack rejects instructions carrying more than one
    sync-wait command.  Hoist surplus waits onto same-engine NoOps inserted
    directly before the instruction (queue order preserves semantics)."""
    nop_id = [0]
    for fn in nc.m.functions:
        for blk in fn.blocks:
            out = []
            for ins in blk.instructions:
                si = ins.sync_info
                waits = list(si.on_wait) if si is not None and si.on_wait else []
                limit = max_waits
                if type(ins).__name__ in ("InstDmaTransposeAnt",):
                    limit = 0
                if len(waits) > limit:
                    keep = waits[len(waits) - limit:] if limit else []
                    for w in waits[:len(waits) - limit]:
                        nop = mybir.InstNoOp(
                            name=f"I-waitnop-{nop_id[0]}", ins=[], outs=[])
                        nop_id[0] += 1
                        nop.engine = ins.engine
                        nop.sync_info = mybir.SyncInfo(on_wait=[w], on_update=[])
                        out.append(nop)
                    ins.sync_info = mybir.SyncInfo(
                        on_wait=keep, on_update=list(si.on_update or []))
                out.append(ins)
            blk.instructions = out


def _build_encoder(split_waits=True):
    nc = bass.Bass()
    ts = bass.ts

    x_in = nc.declare_dram_parameter("x", [S, DIM], F32, isOutput=False)
    w_in = nc.declare_dram_parameter("w_pack", [DIM, 2048], BF16, isOutput=False)
    out_d = nc.declare_dram_parameter("out", [S, DIM], F32, isOutput=True)

    with tile.TileContext(nc) as tc, ExitStack() as ctx:
        singles = ctx.enter_context(tc.tile_pool(name="singles", bufs=1))
        actp = ctx.enter_context(tc.tile_pool(name="actp", bufs=2))
        sm2 = ctx.enter_context(tc.tile_pool(name="sm2", bufs=2))
        sm3 = ctx.enter_context(tc.tile_pool(name="sm3", bufs=3))
        sm5 = ctx.enter_context(tc.tile_pool(name="sm5", bufs=5))
        psA = ctx.enter_context(tc.tile_pool(name="psA", bufs=2, space="PSUM"))
        psB = ctx.enter_context(tc.tile_pool(name="psB", bufs=1, space="PSUM"))
        psC = ctx.enter_context(tc.tile_pool(name="psC", bufs=2, space="PSUM"))

        # ---- constants ----
        ident_bf = singles.tile([128, 128], BF16)
        nc.gpsimd.memset(ident_bf[:], 0.0)
        nc.gpsimd.affine_select(
            out=ident_bf[:], in_=ident_bf[:], compare_op=OP.not_equal,
            fill=1.0, base=0, pattern=[[-1, 128]], channel_multiplier=1)
        ones16 = singles.tile([128, 1], F16)
        nc.vector.memset(ones16[:], 1.0)
        ident1_f32 = singles.tile([1, 1], F32)
        nc.vector.memset(ident1_f32[:], 1.0)
        eps_t = singles.tile([128, 1], F32)
        nc.vector.memset(eps_t[:], LN_EPS)
        zero_t = singles.tile([128, 1], F32)
        nc.vector.memset(zero_t[:], 0.0)

        # ---- load weights (1 DMA, scalar queue) and x (2 DMAs, sync) ----
        w_sb = singles.tile([128, 2048], BF16)
        nc.scalar.dma_start(w_sb[:], w_in[:, :])
        x_r = x_in.rearrange("(i p) d -> p i d", p=128)
        h_all = actp.tile([128, NT, DIM], F32, tag="h", name="h0")
        for g, eng in enumerate((nc.sync, nc.scalar, nc.gpsimd, nc.sync)):
            eng.dma_start(h_all[:, 4 * g:4 * g + 4, :],
                          x_r[:, 4 * g:4 * g + 4, :])

        def w_slice(off, li, width):
            return w_sb[:, off + li * width:off + (li + 1) * width]

        class LN:
            """LayerNorm pipeline: stats fused into producer loops, finish()
            per 8-tile half emits rstd/xhat/transposes/copies."""
            def __init__(self, tag):
                self.tag = tag
                self.mv = sm2.tile([128, NT, 2], F32, tag="ln_mv",
                                   name=tag + "_mv")
                self.rstd = sm2.tile([128, NT], F32, tag="ln_rstd",
                                     name=tag + "_rstd")
                self.xh = sm2.tile([128, NT, DIM], BF16, tag="ln_xh",
                                   name=tag + "_xh")
                self.xT = actp.tile([128, S], BF16, tag="ln_xT",
                                    name=tag + "_xT")

            def stat(self, i, src_ap):
                st6 = sm3.tile([128, 6], F32, tag="ln_st6",
                               name=f"{self.tag}_st6_{i}")
                nc.vector.bn_stats(st6[:], src_ap)
                nc.vector.bn_aggr(self.mv[:, i, :], st6[:])

            def finish_q(self, g, h_src):
                """finish one quarter (4 tiles = one transpose group)."""
                lnv = sm3.tile([128, 4], F32, tag="ln_lnv",
                               name=f"{self.tag}_lnv_{g}")
                nc.scalar.activation(lnv[:], self.mv[:, ts(g, 4), 1],
                                     AF.Ln, bias=eps_t[:], scale=1.0)
                nc.scalar.activation(self.rstd[:, ts(g, 4)], lnv[:],
                                     AF.Exp, bias=zero_t[:], scale=-0.5)
                for i in range(4 * g, 4 * g + 4):
                    nc.gpsimd.tensor_scalar(
                        out=self.xh[:, i, :], in0=h_src[:, i, :],
                        scalar1=self.mv[:, i, 0:1],
                        scalar2=self.rstd[:, i:i + 1],
                        op0=OP.subtract, op1=OP.mult)
                tr = psA.tile([128, 512], BF16, tag="psA",
                              name=f"{self.tag}_tr_{g}")
                for j in range(4):
                    nc.tensor.transpose(tr[:, ts(j, 128)],
                                        self.xh[:, 4 * g + j, :],
                                        ident_bf[:])
                nc.vector.tensor_copy(self.xT[:, ts(g, 512)], tr[:])

            def finish(self, half, h_src):
                self.finish_q(2 * half, h_src)
                self.finish_q(2 * half + 1, h_src)

        # ---------------- layers ----------------
        ln1 = LN("ln1_0")
        for i in range(NT):
            ln1.stat(i, h_all[:, i, :])

        for li in range(L):
            # ===== LN1 finish + edge ops (+ fused LN2 stats), half-wise =====
            xT = ln1.xT
            ln2 = LN(f"ln2_{li}")
            s_all = actp.tile([128, NT, DIM], F32, tag="s", name=f"s_{li}")

            def eop_tile(i, li=li, ln2=ln2, s_all=s_all, xT=xT):
                f_ps = psC.tile([128, 3 * DIM], F32, tag="psC",
                                name=f"f_ps_{li}_{i}")
                nc.tensor.matmul(f_ps[:], xT[:, ts(i, 128)],
                                 w_slice(OFF_EOP, li, 384),
                                 start=True, stop=True)
                f_rl = sm2.tile([128, 3 * DIM], BF16, tag="f_rl",
                                name=f"f_rl_{li}_{i}")
                nc.scalar.activation(f_rl[:], f_ps[:], AF.Relu,
                                     bias=zero_t[:], scale=1.0)
                if i % 2 == 0:
                    s01 = sm3.tile([128, DIM], BF16, tag="s01",
                                   name=f"s01_{li}_{i}")
                    nc.gpsimd.tensor_tensor(out=s01[:], in0=f_rl[:, 0:128],
                                            in1=f_rl[:, 128:256], op=OP.add)
                    nc.gpsimd.tensor_tensor(out=s_all[:, i, :], in0=s01[:],
                                            in1=f_rl[:, 256:384], op=OP.add)
                else:
                    nc.vector.tensor_reduce(
                        s_all[:, i, :],
                        f_rl[:].rearrange("p (j e) -> p e j", j=3),
                        axis=mybir.AxisListType.X, op=OP.add)
                ln2.stat(i, s_all[:, i, :])

            hT = ln2.xT
            q8 = actp.tile([128, S], F8, tag="q8", name=f"q8_{li}")
            k8 = actp.tile([128, S], F8, tag="k8", name=f"k8_{li}")
            v16 = actp.tile([128, S], F16, tag="v16", name=f"v16_{li}")
            # folded [64, 2, S] (partitions 64-127 into plane 1) so the
            # scores matmul can run in fp8 DoubleRow mode (2 K-planes)
            q8p = actp.tile([64, 2, S], F8, tag="q8p", name=f"q8p_{li}")
            k8p = actp.tile([64, 2, S], F8, tag="k8p", name=f"k8p_{li}")

            def qk_one(nm, hb):
                dst, dstp, off = ((q8, q8p, OFF_Q) if nm == "q"
                                  else (k8, k8p, OFF_K))
                qk_ps = psA.tile([128, 1024], F32, tag="psA",
                                 name=f"qk_{nm}_{li}_{hb}")
                for b in range(2):
                    nc.tensor.matmul(
                        qk_ps[:, ts(b, 512)], w_slice(off, li, 128),
                        hT[:, hb * 1024 + b * 512:hb * 1024 + (b + 1) * 512],
                        start=True, stop=True)
                nc.scalar.activation(dst[:, ts(hb, 1024)], qk_ps[:],
                                     AF.Copy, bias=0.0, scale=1.0)
                nc.sync.dma_start(dstp[:, 0, ts(hb, 1024)],
                                  dst[0:64, ts(hb, 1024)])
                nc.sync.dma_start(dstp[:, 1, ts(hb, 1024)],
                                  dst[64:128, ts(hb, 1024)])

            def v_tile(i):
                v_ps = psC.tile([128, DIM], F32, tag="psC",
                                name=f"v_ps_{li}_{i}")
                nc.tensor.matmul(v_ps[:], hT[:, ts(i, 128)],
                                 w_slice(OFF_V, li, 128), start=True, stop=True)
                nc.vector.tensor_copy(v16[:, ts(i, 128)], v_ps[:])

            ln1.finish_q(0, h_all)
            for i in range(8):
                eop_tile(i)
                if i == 0:
                    ln1.finish_q(1, h_all)
                elif i == 2:
                    ln1.finish_q(2, h_all)
                elif i == 4:
                    ln1.finish_q(3, h_all)
            ln2.finish(0, s_all)
            qk_one("q", 0)
            qk_one("k", 0)
            for i in range(8, NT):
                eop_tile(i)

            # ===== attention =====
            r_all = actp.tile([128, NT, DIM], F32, tag="r", name=f"r_{li}")
            ln3 = LN(f"ln3_{li}")

            hb_state = {}

            def emit_att(hb, pj, p16, att_acc, rs, which="both", base=0):
                # att matmuls first (one v-stationary load per tj), then all
                # rs matmuls (one ones-stationary load) — 3 LW per pair
                # instead of 8 on real hardware
                for r in range(2):
                    tj = 2 * pj + r
                    if which in ("both", "att"):
                        for b in range(2):
                            o = base + r * 1024 + b * 512
                            nc.tensor.matmul(att_acc[:, ts(b, 512)],
                                             v16[:, ts(tj, 128)],
                                             p16[:, o:o + 512],
                                             start=(tj == 0),
                                             stop=(tj == NT - 1))
                for r in range(2):
                    tj = 2 * pj + r
                    if which in ("both", "rs"):
                        for b in range(2):
                            o = base + r * 1024 + b * 512
                            nc.tensor.matmul(rs[b][:], ones16[:],
                                             p16[:, o:o + 512],
                                             start=(tj == 0),
                                             stop=(tj == NT - 1))

            def post_hb_a(hb):
                """att^T -> natural + rowsum -> recip.

                hb0 runs during the hb1 attention loop: use DMA transposes
                (PE is busy).  hb1 runs in the layer tail where PE idles:
                copy + PE transposes, fixups read the psum directly."""
                att_acc, rs = hb_state[hb]
                attT = sm2.tile([128, 1024], BF16, tag="attT",
                                name=f"attT_{li}_{hb}")
                nc.vector.tensor_copy(attT[:, 0:512], att_acc[:, 0:512])
                nc.vector.tensor_copy(attT[:, 512:1024], att_acc[:, 512:1024])
                if hb == 0:
                    att_nat = sm2.tile([128, 8, 128], BF16, tag="att_nat",
                                       name=f"att_nat_{li}_{hb}")
                    for k in range(8):
                        nc.sync.dma_start_transpose(att_nat[:, k, :],
                                                    attT[:, ts(k, 128)])
                    nat = [att_nat[:, k, :] for k in range(8)]
                else:
                    nat = []
                    for g in range(2):
                        atr = psA.tile([128, 512], BF16, tag="psA",
                                       name=f"atr_{li}_{hb}_{g}")
                        for j in range(4):
                            nc.tensor.transpose(atr[:, ts(j, 128)],
                                                attT[:, ts(4 * g + j, 128)],
                                                ident_bf[:])
                        nat.extend(atr[:, ts(j, 128)] for j in range(4))
                rs_sb = sm3.tile([1, 1024], F32, tag="rs_sb",
                                 name=f"rs_sb_{li}_{hb}")
                for b in range(2):
                    nc.scalar.activation(rs_sb[:, ts(b, 512)], rs[b][:],
                                         AF.Copy, bias=0.0, scale=1.0)
                rsT_ps = psC.tile([128, 8], F32, tag="psC",
                                  name=f"rsT_{li}_{hb}")
                for k in range(8):
                    nc.tensor.transpose(rsT_ps[:, k:k + 1],
                                        rs_sb[0:1, ts(k, 128)], ident1_f32[:])
                recip = sm3.tile([128, 8], F32, tag="recip",
                                 name=f"recip_{li}_{hb}")
                nc.vector.reciprocal(recip[:], rsT_ps[:])
                hb_state[hb] = (nat, recip)

            def post_hb_b(hb, klo, khi):
                """fixup r = att*recip + s (DVE) + fused LN3 stats."""
                nat, recip = hb_state[hb]
                for k in range(klo, khi):
                    i = hb * 8 + k
                    nc.vector.scalar_tensor_tensor(
                        out=r_all[:, i, :], in0=nat[k],
                        scalar=recip[:, k:k + 1], in1=s_all[:, i, :],
                        op0=OP.mult, op1=OP.add)
                    ln3.stat(i, r_all[:, i, :])

            def _fill_hb0_p0():
                for i in range(0, 8):
                    v_tile(i)

            def _fill_hb0_p1():
                ln2.finish(1, s_all)
                qk_one("k", 1)

            def _fill_hb0_p2():
                qk_one("q", 1)

            def _fill_hb0_p3():
                for i in range(8, NT):
                    v_tile(i)

            extras = {
                0: {0: _fill_hb0_p0, 1: _fill_hb0_p1,
                    2: _fill_hb0_p2, 3: _fill_hb0_p3},
                1: {0: lambda: post_hb_a(0),
                    2: lambda: post_hb_b(0, 0, 2),
                    3: lambda: post_hb_b(0, 2, 4),
                    4: lambda: post_hb_b(0, 4, 6),
                    5: lambda: (post_hb_b(0, 6, 8),
                                ln3.finish_q(0, r_all)),
                    6: lambda: ln3.finish_q(1, r_all)},
            }
            for hb in range(2):
                att_acc = None
                rs = None
                pend_att = []
                pend_rs = []
                for pj in range(8):
                    e16 = sm2.tile([128, 2048], F16, tag="e16",
                                   name=f"e16_{li}_{hb}_{pj}")
                    for r in range(2):
                        tj = 2 * pj + r
                        sc = psA.tile([128, 1024], F32, tag="psA",
                                      name=f"sc_{li}_{hb}_{tj}")
                        for b in range(2):
                            nc.tensor.matmul(
                                sc[:, ts(b, 512)], k8p[:, :, ts(tj, 128)],
                                q8p[:, :, hb * 1024 + b * 512:
                                    hb * 1024 + (b + 1) * 512],
                                start=True, stop=True, perf_mode=DR)
                        nc.scalar.activation(e16[:, ts(r, 1024)], sc[:],
                                             AF.Exp, bias=zero_t[:], scale=1.0)
                    m16 = sm2.tile([128, 2048], F16, tag="m16",
                                   name=f"m16_{li}_{hb}_{pj}")
                    nc.vector.tensor_scalar(out=m16[:], in0=e16[:],
                                            scalar1=CPRIME, scalar2=None,
                                            op0=OP.is_ge)
                    p16 = sm5.tile([128, 2048], F16, tag="p16",
                                   name=f"p16_{li}_{hb}_{pj}")
                    nc.vector.tensor_tensor(out=p16[:], in0=e16[:],
                                            in1=m16[:], op=OP.mult)
                    pend_att.append((pj, p16, 0))
                    pend_rs.append((pj, p16, 0))
                    while len(pend_att) > 2:
                        if att_acc is None:
                            att_acc = psB.tile([128, 1024], F32, tag="att",
                                               name=f"att_{li}_{hb}")
                        j, pt, ba = pend_att.pop(0)
                        emit_att(hb, j, pt, att_acc, None, which="att", base=ba)
                    while len(pend_rs) > 4:
                        if rs is None:
                            rs = [psC.tile([1, 512], F32, tag="psC",
                                           name=f"rs_{li}_{hb}_{b}")
                                  for b in range(2)]
                        j, pt, ba = pend_rs.pop(0)
                        emit_att(hb, j, pt, None, rs, which="rs", base=ba)
                    # interleaved fill work (v/q projections, prev-hb post)
                    fn = extras[hb].get(pj)
                    if fn is not None:
                        fn()
                # drain: att first (its stop gates the attT copy), then rs
                for j, pt, ba in pend_att:
                    emit_att(hb, j, pt, att_acc, None, which="att", base=ba)
                for j, pt, ba in pend_rs:
                    emit_att(hb, j, pt, None, rs, which="rs", base=ba)
                hb_state[hb] = (att_acc, rs)

            # ===== FFN (interleaved with hb1 post-processing) =====
            gT = ln3.xT
            mT = actp.tile([128, S], BF16, tag="mT", name=f"mT_{li}")
            last = (li == L - 1)
            new_h = actp.tile([128, NT, DIM], F32, tag="h",
                              name=f"h{li + 1}")
            if not last:
                ln_next = LN(f"ln1_{li + 1}")
            out_r = out_d.rearrange("(i p) d -> p i d", p=128)

            def ffn1_half(hb):
                m_ps = psA.tile([128, 1024], F32, tag="psA",
                                name=f"m_ps_{li}_{hb}")
                for b in range(2):
                    nc.tensor.matmul(
                        m_ps[:, ts(b, 512)], w_slice(OFF_1, li, 128),
                        gT[:, hb * 1024 + b * 512:hb * 1024 + (b + 1) * 512],
                        start=True, stop=True)
                nc.scalar.activation(mT[:, ts(hb, 1024)], m_ps[:],
                                     AF.Relu, bias=zero_t[:], scale=1.0)

            def ffn1_chunk(hb, b):
                m_ps = psC.tile([128, 512], F32, tag="psC",
                                name=f"m_ps_{li}_{hb}_{b}")
                nc.tensor.matmul(
                    m_ps[:], w_slice(OFF_1, li, 128),
                    gT[:, hb * 1024 + b * 512:hb * 1024 + (b + 1) * 512],
                    start=True, stop=True)
                nc.scalar.activation(mT[:, hb * 1024 + b * 512:
                                        hb * 1024 + (b + 1) * 512], m_ps[:],
                                     AF.Relu, bias=zero_t[:], scale=1.0)

            def ffn2_tile(i):
                h2_ps = psC.tile([128, DIM], F32, tag="psC",
                                 name=f"h2_ps_{li}_{i}")
                nc.tensor.matmul(h2_ps[:], mT[:, ts(i, 128)],
                                 w_slice(OFF_2, li, 128), start=True, stop=True)
                nc.vector.scalar_tensor_tensor(
                    out=new_h[:, i, :], in0=h2_ps[:], scalar=0.0,
                    in1=r_all[:, i, :], op0=OP.bypass, op1=OP.add)
                if not last:
                    ln_next.stat(i, new_h[:, i, :])
                elif i % 4 == 3:
                    nc.sync.dma_start(out_r[:, i - 3:i + 1, :],
                                      new_h[:, i - 3:i + 1, :])

            ffn1_half(0)
            post_hb_a(1)
            for i in range(0, 4):
                ffn2_tile(i)
            post_hb_b(1, 0, 4)
            for i in range(4, 8):
                ffn2_tile(i)
            post_hb_b(1, 4, 8)
            ln3.finish_q(2, r_all)
            ffn1_chunk(1, 0)
            for i in range(8, 12):
                ffn2_tile(i)
            ln3.finish_q(3, r_all)
            ffn1_chunk(1, 1)
            for i in range(12, NT):
                ffn2_tile(i)
            if not last:
                h_all = new_h
                ln1 = ln_next

    if split_waits:
        _split_multi_waits(nc)
    return nc


def _fold_weights(inputs):
    """Fold LN gamma/beta and softmax scale into the linear weights (fp32)."""
    g = {k: np.asarray(v, np.float32) for k, v in inputs.items()}
    scale = 1.0 / math.sqrt(HEAD_SIZE)
    Wp_eop = np.einsum("lod,lode->lode", g["eop_ln_w"], g["eop_W"])
    bp_eop = np.einsum("lod,lode->loe", g["eop_ln_b"], g["eop_W"]) + g["eop_b"]
    Wp_q = np.einsum("ld,lde->lde", g["attn_ln_w"], g["Wq"]) * scale
    bp_q = (np.einsum("ld,lde->le", g["attn_ln_b"], g["Wq"]) + g["bq"]) * scale
    Wp_k = np.einsum("ld,lde->lde", g["attn_ln_w"], g["Wk"])
    bp_k = np.einsum("ld,lde->le", g["attn_ln_b"], g["Wk"]) + g["bk"]
    Wp_v = np.einsum("ld,lde->lde", g["attn_ln_w"], g["Wv"])
    bp_v = np.einsum("ld,lde->le", g["attn_ln_b"], g["Wv"]) + g["bv"]
    Wp_1 = np.einsum("ld,lde->lde", g["ffn_ln_w"], g["W1"])
    bp_1 = np.einsum("ld,lde->le", g["ffn_ln_b"], g["W1"]) + g["b1"]
    biases = [bp_eop, bp_q, bp_k, bp_v, bp_1, g["b2"]]
    w_eop_f = np.concatenate([Wp_eop[:, o] for o in range(3)], axis=-1)
    return (w_eop_f, Wp_q, Wp_k, Wp_v, Wp_1, g["W2"]), biases


def _pack_weights(inputs):
    """Pack all folded weights into one [128, 2048] bf16 array."""
    (w_eop_f, Wp_q, Wp_k, Wp_v, Wp_1, W2), biases = _fold_weights(inputs)
    w = np.zeros((DIM, 2048), np.float32)
    for li in range(L):
        w[:, OFF_EOP + li * 384:OFF_EOP + (li + 1) * 384] = w_eop_f[li]
        w[:, OFF_Q + li * 128:OFF_Q + (li + 1) * 128] = Wp_q[li]
        w[:, OFF_K + li * 128:OFF_K + (li + 1) * 128] = Wp_k[li]
        w[:, OFF_V + li * 128:OFF_V + (li + 1) * 128] = Wp_v[li]
        w[:, OFF_1 + li * 128:OFF_1 + (li + 1) * 128] = Wp_1[li]
        w[:, OFF_2 + li * 128:OFF_2 + (li + 1) * 128] = W2[li]
    return np.ascontiguousarray(w.astype(ml_dtypes.bfloat16)), biases


def _device_inputs(inputs):
    """Per-core name->array maps for the device kernel (fast path only)."""
    w_pack, _ = _pack_weights(inputs)
    x = np.asarray(inputs["x"], np.float32)
    shared = {"w_pack": w_pack}
    return [dict(shared, x=np.ascontiguousarray(x[b])) for b in range(B)]


def _numpy_fallback(inputs):
    """Exact (fp32) host implementation for inputs outside the fast path."""
    ARCH = [[0, 0, 0, 0, 1], [0, 1, 0, 0, 1]]
    g = {k: np.asarray(v, np.float32) for k, v in inputs.items()}
    scale = 1.0 / math.sqrt(HEAD_SIZE)

    def ln(x, w, b):
        u = x.mean(-1, keepdims=True)
        s = ((x - u) ** 2).mean(-1, keepdims=True)
        return w * ((x - u) / np.sqrt(s + LN_EPS)) + b

    def edge(h, li, oi):
        h = ln(h, g["eop_ln_w"][li, oi], g["eop_ln_b"][li, oi])
        return np.maximum(h @ g["eop_W"][li, oi] + g["eop_b"][li, oi], 0.0)

    xs = [g["x"]]
    for i, (o1, prev, o2, o3, n) in enumerate(ARCH):
        s = edge(xs[i], i, 0) + edge(xs[prev], i, 1) + edge(xs[prev], i, 2)
        h = ln(s, g["attn_ln_w"][i], g["attn_ln_b"][i])
        q = h @ g["Wq"][i] + g["bq"][i]
        k = h @ g["Wk"][i] + g["bk"][i]
        v = h @ g["Wv"][i] + g["bv"][i]
        sc = np.einsum("bsd,btd->bst", q, k) * g["mask"] * scale
        sc = np.where(sc < THRESH, np.float32(-10000.0), sc).astype(np.float32)
        sc -= sc.max(axis=2, keepdims=True)
        p = np.exp(sc)
        p /= p.sum(axis=2, keepdims=True)
        att = np.einsum("bst,btd->bsd", p, v) + s
        h2 = ln(att, g["ffn_ln_w"][i], g["ffn_ln_b"][i])
        h2 = np.maximum(h2 @ g["W1"][i] + g["b1"][i], 0.0)
        h2 = h2 @ g["W2"][i] + g["b2"][i]
        xs.append(h2 + att)
    return xs[-1].astype(np.float32)


_LAST_RESULTS = {}


def kernel(**inputs):
    mask = np.asarray(inputs["mask"])
    _, biases = _fold_weights(inputs)

    fast = bool(np.all(mask == 1.0)) and all(
        float(np.abs(b).max()) == 0.0 for b in biases)
    if not fast:
        return _numpy_fallback(inputs)

    if "nc" not in _BUILD_CACHE:
        _BUILD_CACHE["nc"] = _build_encoder()
    nc = _BUILD_CACHE["nc"]

    in_maps = _device_inputs(inputs)
    res = run_bass_kernel_spmd(nc, in_maps, core_ids=list(range(B)),
                               trace=_LAST_RESULTS.get("trace", False))
    _LAST_RESULTS["results"] = res
    return np.stack([res.results[b]["out"] for b in range(B)], axis=0)


# revision 65
# speedup vs baseline: 1.1138x; 1.0085x over previous
"""Trainium2 Bass kernel for nn_Encoder_17824114278582.

Strategy (v2):
- Data-parallel over batch B=8 across 8 NeuronCores (1 batch elem / core).
- Host-side: fold LayerNorm gamma/beta + softmax scale into the linear
  weights; pack ALL weights into one [128, 2048] bf16 DRAM tensor (1 DMA).
  Non-ones mask / non-zero biases fall back to an exact numpy path.
- On-device per layer, engine-balanced + software-pipelined (Pool/GPSIMD
  cannot read PSUM and cannot run scalar_tensor_tensor, so all PSUM
  consumers sit on ACT/DVE; Pool gets SBUF-side work only):
    LN (bn_stats/aggr DVE, rstd ACT per-quarter, xhat Pool, PE transpose,
    psum->sbuf copies DVE 4x-mode), pipelined at 4-tile granularity
    eop: fused [d,384] matmul -> relu (ACT) -> 3-way sum (Pool adds /
    DVE reduce alternating)
    q/k projected to fp8, DMA-folded [128,S]->[64,2,S] so the SxS scores
    matmul runs in fp8 DoubleRow mode (2 K-planes, 2x PE throughput);
    v via hT-stationary matmuls (DVE copies), emitted inside the hb0
    attention loop as fill work
    attention (transposed scores), per tj-pair [128,2048] granularity:
       scores (PE fp8 DoubleRow) -> exp (ACT, fp16) -> mask = e>=c (DVE
       tensor_scalar imm, 4x mode) -> p = e*mask (DVE tensor_tensor, 2x)
       att_acc += v @ p (PE, lag 2 pairs); rs += ones @ p (PE, lag 4)
    rowsum -> partition form via PE transpose + DVE recip; att^T ->
    natural via per-tile DMA transposes (frees PE/PSUM); fixup
    r = att*recip + s (DVE STT); prev-hb post-processing and the FFN
    halves are interleaved into the next attention loop / layer tail
    FFN: W-stationary + relu (ACT) -> mT; ffn2 + residual (DVE STT)
- All input/output/fold/transpose DMAs batched on the idle SP queue.
- Measured (TimelineSim single-core): 258us (v1 baseline) -> 191us.
"""
import sys
for _p in ("/opt/trn_rl_repo", "/root/.axon_site/_ro/trn_rl_repo"):
    if _p not in sys.path:
        sys.path.insert(0, _p)

import math
from contextlib import ExitStack

import numpy as np
import ml_dtypes

import concourse.bass as bass
import concourse.tile as tile
from concourse import mybir
from concourse.bass_utils import run_bass_kernel_spmd

F32 = mybir.dt.float32
BF16 = mybir.dt.bfloat16
F16 = mybir.dt.float16
F8 = mybir.dt.float8e4
AF = mybir.ActivationFunctionType
OP = mybir.AluOpType
DR = mybir.MatmulPerfMode.DoubleRow

B, S, DIM = 8, 2048, 128
L = 2
HEAD_SIZE = 32
NT = S // 128
LN_EPS = 1e-12
THRESH = 1e-3
CPRIME = float(np.float16(np.exp(np.float32(THRESH))))

# column offsets inside the packed weight tensor [128, 2048]
OFF_EOP = 0     # li*384
OFF_Q = 768     # li*128
OFF_K = 1024
OFF_V = 1280
OFF_1 = 1536
OFF_2 = 1792

_BUILD_CACHE = {}


def _split_multi_waits(nc, max_waits=1):
    """walrus on this stack rejects instructions carrying more than one
    sync-wait command.  Hoist surplus waits onto same-engine NoOps inserted
    directly before the instruction (queue order preserves semantics)."""
    nop_id = [0]
    for fn in nc.m.functions:
        for blk in fn.blocks:
            out = []
            for ins in blk.instructions:
                si = ins.sync_info
                waits = list(si.on_wait) if si is not None and si.on_wait else []
                limit = max_waits
                if type(ins).__name__ in ("InstDmaTransposeAnt",):
                    limit = 0
                if len(waits) > limit:
                    keep = waits[len(waits) - limit:] if limit else []
                    for w in waits[:len(waits) - limit]:
                        nop = mybir.InstNoOp(
                            name=f"I-waitnop-{nop_id[0]}", ins=[], outs=[])
                        nop_id[0] += 1
                        nop.engine = ins.engine
                        nop.sync_info = mybir.SyncInfo(on_wait=[w], on_update=[])
                        out.append(nop)
                    ins.sync_info = mybir.SyncInfo(
                        on_wait=keep, on_update=list(si.on_update or []))
                out.append(ins)
            blk.instructions = out


def _build_encoder(split_waits=True):
    nc = bass.Bass()
    ts = bass.ts

    x_in = nc.declare_dram_parameter("x", [S, DIM], F32, isOutput=False)
    w_in = nc.declare_dram_parameter("w_pack", [DIM, 2048], BF16, isOutput=False)
    out_d = nc.declare_dram_parameter("out", [S, DIM], F32, isOutput=True)

    with tile.TileContext(nc) as tc, ExitStack() as ctx:
        singles = ctx.enter_context(tc.tile_pool(name="singles", bufs=1))
        actp = ctx.enter_context(tc.tile_pool(name="actp", bufs=2))
        sm2 = ctx.enter_context(tc.tile_pool(name="sm2", bufs=2))
        sm3 = ctx.enter_context(tc.tile_pool(name="sm3", bufs=3))
        sm5 = ctx.enter_context(tc.tile_pool(name="sm5", bufs=5))
        psA = ctx.enter_context(tc.tile_pool(name="psA", bufs=2, space="PSUM"))
        psB = ctx.enter_context(tc.tile_pool(name="psB", bufs=1, space="PSUM"))
        psC = ctx.enter_context(tc.tile_pool(name="psC", bufs=2, space="PSUM"))

        # ---- constants ----
        ident_bf = singles.tile([128, 128], BF16)
        nc.gpsimd.memset(ident_bf[:], 0.0)
        nc.gpsimd.affine_select(
            out=ident_bf[:], in_=ident_bf[:], compare_op=OP.not_equal,
            fill=1.0, base=0, pattern=[[-1, 128]], channel_multiplier=1)
        ones16 = singles.tile([128, 1], F16)
        nc.vector.memset(ones16[:], 1.0)
        ident1_f32 = singles.tile([1, 1], F32)
        nc.vector.memset(ident1_f32[:], 1.0)
        eps_t = singles.tile([128, 1], F32)
        nc.vector.memset(eps_t[:], LN_EPS)
        zero_t = singles.tile([128, 1], F32)
        nc.vector.memset(zero_t[:], 0.0)

        # ---- load weights (1 DMA, scalar queue) and x (2 DMAs, sync) ----
        w_sb = singles.tile([128, 2048], BF16)
        nc.scalar.dma_start(w_sb[:], w_in[:, :])
        x_r = x_in.rearrange("(i p) d -> p i d", p=128)
        h_all = actp.tile([128, NT, DIM], F32, tag="h", name="h0")
        for g, eng in enumerate((nc.sync, nc.scalar, nc.gpsimd, nc.sync)):
            eng.dma_start(h_all[:, 4 * g:4 * g + 4, :],
                          x_r[:, 4 * g:4 * g + 4, :])

        def w_slice(off, li, width):
            return w_sb[:, off + li * width:off + (li + 1) * width]

        class LN:
            """LayerNorm pipeline: stats fused into producer loops, finish()
            per 8-tile half emits rstd/xhat/transposes/copies."""
            def __init__(self, tag):
                self.tag = tag
                self.mv = sm2.tile([128, NT, 2], F32, tag="ln_mv",
                                   name=tag + "_mv")
                self.rstd = sm2.tile([128, NT], F32, tag="ln_rstd",
                                     name=tag + "_rstd")
                self.xh = sm2.tile([128, NT, DIM], BF16, tag="ln_xh",
                                   name=tag + "_xh")
                self.xT = actp.tile([128, S], BF16, tag="ln_xT",
                                    name=tag + "_xT")

            def stat(self, i, src_ap):
                st6 = sm3.tile([128, 6], F32, tag="ln_st6",
                               name=f"{self.tag}_st6_{i}")
                nc.vector.bn_stats(st6[:], src_ap)
                nc.vector.bn_aggr(self.mv[:, i, :], st6[:])

            def finish_q(self, g, h_src):
                """finish one quarter (4 tiles = one transpose group)."""
                lnv = sm3.tile([128, 4], F32, tag="ln_lnv",
                               name=f"{self.tag}_lnv_{g}")
                nc.scalar.activation(lnv[:], self.mv[:, ts(g, 4), 1],
                                     AF.Ln, bias=eps_t[:], scale=1.0)
                nc.scalar.activation(self.rstd[:, ts(g, 4)], lnv[:],
                                     AF.Exp, bias=zero_t[:], scale=-0.5)
                for i in range(4 * g, 4 * g + 4):
                    nc.gpsimd.tensor_scalar(
                        out=self.xh[:, i, :], in0=h_src[:, i, :],
                        scalar1=self.mv[:, i, 0:1],
                        scalar2=self.rstd[:, i:i + 1],
                        op0=OP.subtract, op1=OP.mult)
                tr = psA.tile([128, 512], BF16, tag="psA",
                              name=f"{self.tag}_tr_{g}")
                for j in range(4):
                    nc.tensor.transpose(tr[:, ts(j, 128)],
                                        self.xh[:, 4 * g + j, :],
                                        ident_bf[:])
                nc.vector.tensor_copy(self.xT[:, ts(g, 512)], tr[:])

            def finish(self, half, h_src):
                self.finish_q(2 * half, h_src)
                self.finish_q(2 * half + 1, h_src)

        # ---------------- layers ----------------
        ln1 = LN("ln1_0")
        for i in range(NT):
            ln1.stat(i, h_all[:, i, :])

        for li in range(L):
            # ===== LN1 finish + edge ops (+ fused LN2 stats), half-wise =====
            xT = ln1.xT
            ln2 = LN(f"ln2_{li}")
            s_all = actp.tile([128, NT, DIM], F32, tag="s", name=f"s_{li}")

            def eop_tile(i, li=li, ln2=ln2, s_all=s_all, xT=xT):
                f_ps = psC.tile([128, 3 * DIM], F32, tag="psC",
                                name=f"f_ps_{li}_{i}")
                nc.tensor.matmul(f_ps[:], xT[:, ts(i, 128)],
                                 w_slice(OFF_EOP, li, 384),
                                 start=True, stop=True)
                f_rl = sm2.tile([128, 3 * DIM], BF16, tag="f_rl",
                                name=f"f_rl_{li}_{i}")
                nc.scalar.activation(f_rl[:], f_ps[:], AF.Relu,
                                     bias=zero_t[:], scale=1.0)
                if i % 2 == 0:
                    s01 = sm3.tile([128, DIM], BF16, tag="s01",
                                   name=f"s01_{li}_{i}")
                    nc.gpsimd.tensor_tensor(out=s01[:], in0=f_rl[:, 0:128],
                                            in1=f_rl[:, 128:256], op=OP.add)
                    nc.gpsimd.tensor_tensor(out=s_all[:, i, :], in0=s01[:],
                                            in1=f_rl[:, 256:384], op=OP.add)
                else:
                    nc.vector.tensor_reduce(
                        s_all[:, i, :],
                        f_rl[:].rearrange("p (j e) -> p e j", j=3),
                        axis=mybir.AxisListType.X, op=OP.add)
                ln2.stat(i, s_all[:, i, :])

            hT = ln2.xT
            q8 = actp.tile([128, S], F8, tag="q8", name=f"q8_{li}")
            k8 = actp.tile([128, S], F8, tag="k8", name=f"k8_{li}")
            v16 = actp.tile([128, S], F16, tag="v16", name=f"v16_{li}")
            # folded [64, 2, S] (partitions 64-127 into plane 1) so the
            # scores matmul can run in fp8 DoubleRow mode (2 K-planes)
            q8p = actp.tile([64, 2, S], F8, tag="q8p", name=f"q8p_{li}")
            k8p = actp.tile([64, 2, S], F8, tag="k8p", name=f"k8p_{li}")

            def qk_one(nm, hb):
                dst, dstp, off = ((q8, q8p, OFF_Q) if nm == "q"
                                  else (k8, k8p, OFF_K))
                qk_ps = psA.tile([128, 1024], F32, tag="psA",
                                 name=f"qk_{nm}_{li}_{hb}")
                for b in range(2):
                    nc.tensor.matmul(
                        qk_ps[:, ts(b, 512)], w_slice(off, li, 128),
                        hT[:, hb * 1024 + b * 512:hb * 1024 + (b + 1) * 512],
                        start=True, stop=True)
                nc.scalar.activation(dst[:, ts(hb, 1024)], qk_ps[:],
                                     AF.Copy, bias=0.0, scale=1.0)
                nc.sync.dma_start(dstp[:, 0, ts(hb, 1024)],
                                  dst[0:64, ts(hb, 1024)])
                nc.sync.dma_start(dstp[:, 1, ts(hb, 1024)],
                                  dst[64:128, ts(hb, 1024)])

            def v_tile(i):
                v_ps = psC.tile([128, DIM], F32, tag="psC",
                                name=f"v_ps_{li}_{i}")
                nc.tensor.matmul(v_ps[:], hT[:, ts(i, 128)],
                                 w_slice(OFF_V, li, 128), start=True, stop=True)
                nc.vector.tensor_copy(v16[:, ts(i, 128)], v_ps[:])

            ln1.finish_q(0, h_all)
            for i in range(8):
                eop_tile(i)
                if i == 0:
                    ln1.finish_q(1, h_all)
                elif i == 2:
                    ln1.finish_q(2, h_all)
                elif i == 4:
                    ln1.finish_q(3, h_all)
            ln2.finish(0, s_all)
            qk_one("q", 0)
            qk_one("k", 0)
            for i in range(8, NT):
                eop_tile(i)

            # ===== attention =====
            r_all = actp.tile([128, NT, DIM], F32, tag="r", name=f"r_{li}")
            ln3 = LN(f"ln3_{li}")

            hb_state = {}

            def emit_att(hb, pj, p16, att_acc, rs, which="both", base=0):
                # att matmuls first (one v-stationary load per tj), then all
                # rs matmuls (one ones-stationary load) — 3 LW per pair
                # instead of 8 on real hardware
                for r in range(2):
                    tj = 2 * pj + r
                    if which in ("both", "att"):
                        for b in range(2):
                            o = base + r * 1024 + b * 512
                            nc.tensor.matmul(att_acc[:, ts(b, 512)],
                                             v16[:, ts(tj, 128)],
                                             p16[:, o:o + 512],
                                             start=(tj == 0),
                                             stop=(tj == NT - 1))
                for r in range(2):
                    tj = 2 * pj + r
                    if which in ("both", "rs"):
                        for b in range(2):
                            o = base + r * 1024 + b * 512
                            nc.tensor.matmul(rs[b][:], ones16[:],
                                             p16[:, o:o + 512],
                                             start=(tj == 0),
                                             stop=(tj == NT - 1))

            def post_hb_a(hb):
                """att^T -> natural + rowsum -> recip.

                hb0 runs during the hb1 attention loop: use DMA transposes
                (PE is busy).  hb1 runs in the layer tail where PE idles:
                copy + PE transposes, fixups read the psum directly."""
                att_acc, rs = hb_state[hb]
                attT = sm2.tile([128, 1024], BF16, tag="attT",
                                name=f"attT_{li}_{hb}")
                nc.vector.tensor_copy(attT[:, 0:512], att_acc[:, 0:512])
                nc.vector.tensor_copy(attT[:, 512:1024], att_acc[:, 512:1024])
                if hb == 0:
                    att_nat = sm2.tile([128, 8, 128], BF16, tag="att_nat",
                                       name=f"att_nat_{li}_{hb}")
                    for k in range(8):
                        nc.sync.dma_start_transpose(att_nat[:, k, :],
                                                    attT[:, ts(k, 128)])
                    nat = [att_nat[:, k, :] for k in range(8)]
                else:
                    nat = []
                    for g in range(2):
                        atr = psA.tile([128, 512], BF16, tag="psA",
                                       name=f"atr_{li}_{hb}_{g}")
                        for j in range(4):
                            nc.tensor.transpose(atr[:, ts(j, 128)],
                                                attT[:, ts(4 * g + j, 128)],
                                                ident_bf[:])
                        nat.extend(atr[:, ts(j, 128)] for j in range(4))
                rs_sb = sm3.tile([1, 1024], F32, tag="rs_sb",
                                 name=f"rs_sb_{li}_{hb}")
                nc.scalar.activation(rs_sb[:, 0:512], rs[0][:],
                                     AF.Copy, bias=0.0, scale=1.0)
                nc.vector.tensor_copy(rs_sb[:, 512:1024], rs[1][:])
                rsT_ps = psC.tile([128, 8], F32, tag="psC",
                                  name=f"rsT_{li}_{hb}")
                for k in range(8):
                    nc.tensor.transpose(rsT_ps[:, k:k + 1],
                                        rs_sb[0:1, ts(k, 128)], ident1_f32[:])
                recip = sm3.tile([128, 8], F32, tag="recip",
                                 name=f"recip_{li}_{hb}")
                nc.vector.reciprocal(recip[:], rsT_ps[:])
                hb_state[hb] = (nat, recip)

            def post_hb_b(hb, klo, khi):
                """fixup r = att*recip + s (DVE) + fused LN3 stats."""
                nat, recip = hb_state[hb]
                for k in range(klo, khi):
                    i = hb * 8 + k
                    nc.vector.scalar_tensor_tensor(
                        out=r_all[:, i, :], in0=nat[k],
                        scalar=recip[:, k:k + 1], in1=s_all[:, i, :],
                        op0=OP.mult, op1=OP.add)
                    ln3.stat(i, r_all[:, i, :])

            def _fill_hb0_p0():
                for i in range(0, 8):
                    v_tile(i)

            def _fill_hb0_p1():
                ln2.finish(1, s_all)
                qk_one("k", 1)

            def _fill_hb0_p2():
                qk_one("q", 1)

            def _fill_hb0_p3():
                for i in range(8, NT):
                    v_tile(i)

            extras = {
                0: {0: _fill_hb0_p0, 1: _fill_hb0_p1,
                    2: _fill_hb0_p2, 3: _fill_hb0_p3},
                1: {0: lambda: post_hb_a(0),
                    2: lambda: post_hb_b(0, 0, 2),
                    3: lambda: post_hb_b(0, 2, 4),
                    4: lambda: post_hb_b(0, 4, 6),
                    5: lambda: (post_hb_b(0, 6, 8),
                                ln3.finish_q(0, r_all)),
                    6: lambda: ln3.finish_q(1, r_all)},
            }
            for hb in range(2):
                att_acc = None
                rs = None
                pend_att = []
                pend_rs = []
                for pj in range(8):
                    e16 = sm2.tile([128, 2048], F16, tag="e16",
                                   name=f"e16_{li}_{hb}_{pj}")
                    for r in range(2):
                        tj = 2 * pj + r
                        sc = psA.tile([128, 1024], F32, tag="psA",
                                      name=f"sc_{li}_{hb}_{tj}")
                        for b in range(2):
                            nc.tensor.matmul(
                                sc[:, ts(b, 512)], k8p[:, :, ts(tj, 128)],
                                q8p[:, :, hb * 1024 + b * 512:
                                    hb * 1024 + (b + 1) * 512],
                                start=True, stop=True, perf_mode=DR)
                        nc.scalar.activation(e16[:, ts(r, 1024)], sc[:],
                                             AF.Exp, bias=zero_t[:], scale=1.0)
                    m16 = sm2.tile([128, 2048], F16, tag="m16",
                                   name=f"m16_{li}_{hb}_{pj}")
                    nc.vector.tensor_scalar(out=m16[:], in0=e16[:],
                                            scalar1=CPRIME, scalar2=None,
                                            op0=OP.is_ge)
                    p16 = sm5.tile([128, 2048], F16, tag="p16",
                                   name=f"p16_{li}_{hb}_{pj}")
                    nc.vector.tensor_tensor(out=p16[:], in0=e16[:],
                                            in1=m16[:], op=OP.mult)
                    pend_att.append((pj, p16, 0))
                    pend_rs.append((pj, p16, 0))
                    while len(pend_att) > 2:
                        if att_acc is None:
                            att_acc = psB.tile([128, 1024], F32, tag="att",
                                               name=f"att_{li}_{hb}")
                        j, pt, ba = pend_att.pop(0)
                        emit_att(hb, j, pt, att_acc, None, which="att", base=ba)
                    while len(pend_rs) > 4:
                        if rs is None:
                            rs = [psC.tile([1, 512], F32, tag="psC",
                                           name=f"rs_{li}_{hb}_{b}")
                                  for b in range(2)]
                        j, pt, ba = pend_rs.pop(0)
                        emit_att(hb, j, pt, None, rs, which="rs", base=ba)
                    # interleaved fill work (v/q projections, prev-hb post)
                    fn = extras[hb].get(pj)
                    if fn is not None:
                        fn()
                # drain: att first (its stop gates the attT copy), then rs
                for j, pt, ba in pend_att:
                    emit_att(hb, j, pt, att_acc, None, which="att", base=ba)
                for j, pt, ba in pend_rs:
                    emit_att(hb, j, pt, None, rs, which="rs", base=ba)
                hb_state[hb] = (att_acc, rs)

            # ===== FFN (interleaved with hb1 post-processing) =====
            gT = ln3.xT
            mT = actp.tile([128, S], BF16, tag="mT", name=f"mT_{li}")
            last = (li == L - 1)
            new_h = actp.tile([128, NT, DIM], F32, tag="h",
                              name=f"h{li + 1}")
            if not last:
                ln_next = LN(f"ln1_{li + 1}")
            out_r = out_d.rearrange("(i p) d -> p i d", p=128)

            def ffn1_half(hb):
                m_ps = psA.tile([128, 1024], F32, tag="psA",
                                name=f"m_ps_{li}_{hb}")
                for b in range(2):
                    nc.tensor.matmul(
                        m_ps[:, ts(b, 512)], w_slice(OFF_1, li, 128),
                        gT[:, hb * 1024 + b * 512:hb * 1024 + (b + 1) * 512],
                        start=True, stop=True)
                nc.scalar.activation(mT[:, ts(hb, 1024)], m_ps[:],
                                     AF.Relu, bias=zero_t[:], scale=1.0)

            def ffn1_chunk(hb, b):
                m_ps = psC.tile([128, 512], F32, tag="psC",
                                name=f"m_ps_{li}_{hb}_{b}")
                nc.tensor.matmul(
                    m_ps[:], w_slice(OFF_1, li, 128),
                    gT[:, hb * 1024 + b * 512:hb * 1024 + (b + 1) * 512],
                    start=True, stop=True)
                nc.scalar.activation(mT[:, hb * 1024 + b * 512:
                                        hb * 1024 + (b + 1) * 512], m_ps[:],
                                     AF.Relu, bias=zero_t[:], scale=1.0)

            def ffn2_tile(i):
                h2_ps = psC.tile([128, DIM], F32, tag="psC",
                                 name=f"h2_ps_{li}_{i}")
                nc.tensor.matmul(h2_ps[:], mT[:, ts(i, 128)],
                                 w_slice(OFF_2, li, 128), start=True, stop=True)
                nc.vector.scalar_tensor_tensor(
                    out=new_h[:, i, :], in0=h2_ps[:], scalar=0.0,
                    in1=r_all[:, i, :], op0=OP.bypass, op1=OP.add)
                if not last:
                    ln_next.stat(i, new_h[:, i, :])
                elif i % 4 == 3:
                    nc.sync.dma_start(out_r[:, i - 3:i + 1, :],
                                      new_h[:, i - 3:i + 1, :])

            ffn1_half(0)
            post_hb_a(1)
            for i in range(0, 4):
                ffn2_tile(i)
            post_hb_b(1, 0, 4)
            for i in range(4, 8):
                ffn2_tile(i)
            post_hb_b(1, 4, 8)
            ln3.finish_q(2, r_all)
            ffn1_chunk(1, 0)
            for i in range(8, 12):
                ffn2_tile(i)
            ln3.finish_q(3, r_all)
            ffn1_chunk(1, 1)
            for i in range(12, NT):
                ffn2_tile(i)
            if not last:
                h_all = new_h
                ln1 = ln_next

    if split_waits:
        _split_multi_waits(nc)
    return nc


def _fold_weights(inputs):
    """Fold LN gamma/beta and softmax scale into the linear weights (fp32)."""
    g = {k: np.asarray(v, np.float32) for k, v in inputs.items()}
    scale = 1.0 / math.sqrt(HEAD_SIZE)
    Wp_eop = np.einsum("lod,lode->lode", g["eop_ln_w"], g["eop_W"])
    bp_eop = np.einsum("lod,lode->loe", g["eop_ln_b"], g["eop_W"]) + g["eop_b"]
    Wp_q = np.einsum("ld,lde->lde", g["attn_ln_w"], g["Wq"]) * scale
    bp_q = (np.einsum("ld,lde->le", g["attn_ln_b"], g["Wq"]) + g["bq"]) * scale
    Wp_k = np.einsum("ld,lde->lde", g["attn_ln_w"], g["Wk"])
    bp_k = np.einsum("ld,lde->le", g["attn_ln_b"], g["Wk"]) + g["bk"]
    Wp_v = np.einsum("ld,lde->lde", g["attn_ln_w"], g["Wv"])
    bp_v = np.einsum("ld,lde->le", g["attn_ln_b"], g["Wv"]) + g["bv"]
    Wp_1 = np.einsum("ld,lde->lde", g["ffn_ln_w"], g["W1"])
    bp_1 = np.einsum("ld,lde->le", g["ffn_ln_b"], g["W1"]) + g["b1"]
    biases = [bp_eop, bp_q, bp_k, bp_v, bp_1, g["b2"]]
    w_eop_f = np.concatenate([Wp_eop[:, o] for o in range(3)], axis=-1)
    return (w_eop_f, Wp_q, Wp_k, Wp_v, Wp_1, g["W2"]), biases


def _pack_weights(inputs):
    """Pack all folded weights into one [128, 2048] bf16 array."""
    (w_eop_f, Wp_q, Wp_k, Wp_v, Wp_1, W2), biases = _fold_weights(inputs)
    w = np.zeros((DIM, 2048), np.float32)
    for li in range(L):
        w[:, OFF_EOP + li * 384:OFF_EOP + (li + 1) * 384] = w_eop_f[li]
        w[:, OFF_Q + li * 128:OFF_Q + (li + 1) * 128] = Wp_q[li]
        w[:, OFF_K + li * 128:OFF_K + (li + 1) * 128] = Wp_k[li]
        w[:, OFF_V + li * 128:OFF_V + (li + 1) * 128] = Wp_v[li]
        w[:, OFF_1 + li * 128:OFF_1 + (li + 1) * 128] = Wp_1[li]
        w[:, OFF_2 + li * 128:OFF_2 + (li + 1) * 128] = W2[li]
    return np.ascontiguousarray(w.astype(ml_dtypes.bfloat16)), biases


def _device_inputs(inputs):
    """Per-core name->array maps for the device kernel (fast path only)."""
    w_pack, _ = _pack_weights(inputs)
    x = np.asarray(inputs["x"], np.float32)
    shared = {"w_pack": w_pack}
    return [dict(shared, x=np.ascontiguousarray(x[b])) for b in range(B)]


def _numpy_fallback(inputs):
    """Exact (fp32) host implementation for inputs outside the fast path."""
    ARCH = [[0, 0, 0, 0, 1], [0, 1, 0, 0, 1]]
    g = {k: np.asarray(v, np.float32) for k, v in inputs.items()}
    scale = 1.0 / math.sqrt(HEAD_SIZE)

    def ln(x, w, b):
        u = x.mean(-1, keepdims=True)
        s = ((x - u) ** 2).mean(-1, keepdims=True)
        return w * ((x - u) / np.sqrt(s + LN_EPS)) + b

    def edge(h, li, oi):
        h = ln(h, g["eop_ln_w"][li, oi], g["eop_ln_b"][li, oi])
        return np.maximum(h @ g["eop_W"][li, oi] + g["eop_b"][li, oi], 0.0)

    xs = [g["x"]]
    for i, (o1, prev, o2, o3, n) in enumerate(ARCH):
        s = edge(xs[i], i, 0) + edge(xs[prev], i, 1) + edge(xs[prev], i, 2)
        h = ln(s, g["attn_ln_w"][i], g["attn_ln_b"][i])
        q = h @ g["Wq"][i] + g["bq"][i]
        k = h @ g["Wk"][i] + g["bk"][i]
        v = h @ g["Wv"][i] + g["bv"][i]
        sc = np.einsum("bsd,btd->bst", q, k) * g["mask"] * scale
        sc = np.where(sc < THRESH, np.float32(-10000.0), sc).astype(np.float32)
        sc -= sc.max(axis=2, keepdims=True)
        p = np.exp(sc)
        p /= p.sum(axis=2, keepdims=True)
        att = np.einsum("bst,btd->bsd", p, v) + s
        h2 = ln(att, g["ffn_ln_w"][i], g["ffn_ln_b"][i])
        h2 = np.maximum(h2 @ g["W1"][i] + g["b1"][i], 0.0)
        h2 = h2 @ g["W2"][i] + g["b2"][i]
        xs.append(h2 + att)
    return xs[-1].astype(np.float32)


_LAST_RESULTS = {}


def kernel(**inputs):
    mask = np.asarray(inputs["mask"])
    _, biases = _fold_weights(inputs)

    fast = bool(np.all(mask == 1.0)) and all(
        float(np.abs(b).max()) == 0.0 for b in biases)
    if not fast:
        return _numpy_fallback(inputs)

    if "nc" not in _BUILD_CACHE:
        _BUILD_CACHE["nc"] = _build_encoder()
    nc = _BUILD_CACHE["nc"]

    in_maps = _device_inputs(inputs)
    res = run_bass_kernel_spmd(nc, in_maps, core_ids=list(range(B)),
                               trace=_LAST_RESULTS.get("trace", False))
    _LAST_RESULTS["results"] = res
    return np.stack([res.results[b]["out"] for b in range(B)], axis=0)
